# revision 54
# baseline (speedup 1.0000x reference)
"""Locally-connected layer (unshared 3x3 conv, torch-unfold semantics) on 8 trn2 cores.

out[b,o,y,x] = sum_{c,i,j} weight[o, c*9+i*3+j, y*32+x] * xpad[b, c, y+i, x+j]

Sharding: spatial over L — core r owns image rows [4r, 4r+4) (128 pixels).

Design (bf16, weights-stationary, N=128 moving, row-major slab):
  * Everything bf16 on the wire (tolerance 2e-2; measured error ~0.29%).
    PSUM accumulates fp32.
  * SBUF slab T1 [128, (row, b, w)] = [slab | slab shifted +1 col], host-built
    row-major and streamed per slab row, so compute starts as soon as row 0
    plus the first weight group land (PE warmup matmuls run meanwhile).
  * The 576-long contraction is reordered into 6 chunks; the stationary of
    each is a [K, 128] host-packed weight block covering BOTH pixels of a
    pair (cols m = 64*e + o).  Moving operand is x [K, N=128=(b, pix)] read
    as t1r[:, row, :, x0:x0+2].  PSUM [128, (b,pix)]: only the e==pix halves
    are read out.
      q0..q2: K=128  rows [c x (i=q,j=0) | c x (i=q,j=1)]
      s0..s2: K=64   rows  c x (s,2)  (ws blob, rows 0:64)
    All matmuls sit at row base 0 (mixed-base accumulation groups crash TRN2).
  * 6 matmuls / 6 ldweights per pixel pair (384 per core), 128-col
    stationaries, N=128 moving.  PSUM readout alternates vector/scalar by
    pair so the two engines touch different PSUM banks.
  * DMA budget and the throttled PE clock (~1.2 GHz sustained under 8-core
    load) are the binding resources: total moved = 3.34 (x) + 9.44 (w) +
    1.05 (out bf16) MB.  The weight stream owns the sync HWDGE queue
    (nothing ever blocks it); x and output DMAs ride the scalar queue.
  * Output bf16 in [psum-partition, pair, b] DRAM layout (contiguous DMA);
    host transposes to (B, O, H, W) fp32.
"""

import numpy as np
import ml_dtypes

BF16 = ml_dtypes.bfloat16

B, C, O, H, W, KS = 64, 64, 64, 32, 32, 3
L = H * W
NCORES = 8
RPC = H // NCORES            # image rows per core = 4
LC = RPC * W                 # pixels per core = 128
NP = LC // 2                 # pixel pairs per core = 64
HALO = RPC + 2               # 6 slab rows
WP = W + 2                   # padded width 34
PG = 8                       # pairs per weight DMA group
NG = NP // PG                # weight groups = 8

_CACHE = {}


def _build_nc():
    import concourse.bass as bass
    import concourse.bacc as bacc
    import concourse.tile as tile
    from concourse import mybir

    f32 = mybir.dt.float32
    bf16 = mybir.dt.bfloat16
    nc = bacc.Bacc(
        "TRN2", target_bir_lowering=False, debug=False, num_devices=NCORES
    )
    x_d = nc.dram_tensor("x", [128, HALO, B, WP], bf16, kind="ExternalInput")
    wq_d = nc.dram_tensor("wq", [128, NP, 3, 128], bf16, kind="ExternalInput")
    ws_d = nc.dram_tensor("ws", [64, NP, 3, 128], bf16, kind="ExternalInput")
    o_d = nc.dram_tensor("out", [128, NP, B], bf16, kind="ExternalOutput")

    with tile.TileContext(nc) as tc:
        with (
            tc.tile_pool(name="x1", bufs=1) as x1pool,
            tc.tile_pool(name="wq", bufs=6) as wpool,
            tc.tile_pool(name="ws", bufs=6) as spool,
            tc.tile_pool(name="orow", bufs=3) as opool,
            tc.tile_pool(name="ps", bufs=7, space=bass.MemorySpace.PSUM) as pspool,
            tc.tile_pool(name="psw", bufs=1, space=bass.MemorySpace.PSUM) as pswpool,
        ):
            t1 = x1pool.tile([128, HALO * B * WP], bf16)
            t1r = t1[:].rearrange("p (r b w) -> p r b w", r=HALO, b=B)
            # row-major slab: row 0 lands first (pair 0's q0 only needs it),
            # compute starts under the tail of the x transfer.
            nc.scalar.dma_start(t1r[:, 0:1], x_d[:, 0:1])
            nc.scalar.dma_start(t1r[:, 1:2], x_d[:, 1:2])
            nc.scalar.dma_start(t1r[:, 2:3], x_d[:, 2:3])
            nc.scalar.dma_start(t1r[:, 3:6], x_d[:, 3:6])

            # PE warmup: dummy matmuls while the first DMAs stream, so the
            # HAM clock gate is at 8/8 when the real matmul stream begins.
            scr = x1pool.tile([128, 256], bf16)
            nc.vector.memzero(scr[:])
            psw = pswpool.tile([64, 256], f32)
            for _ in range(16):
                nc.tensor.matmul(psw[:], scr[:, 0:64], scr[:], start=True, stop=True)

            # variable-size weight groups: small head (compute starts sooner)
            # and small tail (last output flushes sooner)
            sizes = [4, 4, 8, 8, 8, 8, 8, 8, 4, 4]
            t0g = 0
            for cnt in sizes:
                g0, g1 = t0g, t0g + cnt
                t0g = g1
                wt = wpool.tile([128, cnt, 3, 128], bf16)
                st = spool.tile([64, cnt, 3, 128], bf16)
                nc.sync.dma_start(wt[:], wq_d[:, g0:g1])
                nc.sync.dma_start(st[:], ws_d[:, g0:g1])
                orow = opool.tile([128, cnt, B], bf16)
                for tt in range(cnt):
                    t = g0 + tt
                    y, x0 = (2 * t) // W, (2 * t) % W
                    ps = pspool.tile([128, B, 2], f32)
                    for q in range(3):
                        nc.tensor.matmul(
                            ps[:], wt[:, tt, q, :],
                            t1r[:, y + q, :, x0 : x0 + 2],
                            start=(q == 0), stop=False,
                        )
                    for s in range(3):
                        nc.tensor.matmul(
                            ps[:], st[:, tt, s, :],
                            t1r[0:64, y + s, :, x0 + 2 : x0 + 4],
                            start=False, stop=(s == 2),
                        )
                    nc.vector.tensor_copy(orow[0:64, tt, :], ps[0:64, :, 0])
                    nc.vector.tensor_copy(orow[64:128, tt, :], ps[64:128, :, 1])
                nc.scalar.dma_start(o_d[:, g0:g1, :], orow[:])
    nc.compile()
    return nc


def _get_nc():
    if "nc" not in _CACHE:
        _CACHE["nc"] = _build_nc()
    return _CACHE["nc"]


def _pack_x(x):
    """Per core: [128, HALO, B, WP] bf16 = [slab | slab shifted +1 col],
    row-major so row blocks stream independently."""
    xpad = np.pad(x, ((0, 0), (0, 0), (1, 1), (1, 1)))
    xpad = np.ascontiguousarray(xpad.transpose(1, 0, 2, 3))  # [C, B, 34, 34]
    outs = []
    for r in range(NCORES):
        slab = xpad[:, :, RPC * r : RPC * r + HALO, :]       # [C, B, 6, 34]
        slab = slab.transpose(0, 2, 1, 3)                    # [C, 6, B, 34]
        up = np.zeros_like(slab)
        up[..., : WP - 1] = slab[..., 1:]
        t1 = np.concatenate([slab, up], axis=0).astype(BF16)
        outs.append(np.ascontiguousarray(t1))
    return outs


def _pack_w(weight):
    """Chunked-contraction weight blobs, already in SBUF layout.

    wq: [core, NG, p=(j, c), tt, q, m=(e, o)]   (pair chunks, shifts (q, j))
    ws: [core, NG, c, tt, s, m=(e, o)]          (singles, shifts (s, 2))
    """
    w5 = weight.reshape(O, C, KS, KS, L)
    low = np.stack([w5[:, :, 0, 0], w5[:, :, 1, 0], w5[:, :, 2, 0]], axis=0)
    up = np.stack([w5[:, :, 0, 1], w5[:, :, 1, 1], w5[:, :, 2, 1]], axis=0)
    wq = np.stack([low, up], axis=1)          # [q, j, O, C, L]
    wq = wq.reshape(3, 2, O, C, NCORES, NP, 2)
    # -> [core, j, c, t, q, e, o]
    wq = wq.transpose(4, 1, 3, 5, 0, 6, 2)
    wq = np.ascontiguousarray(wq, dtype=BF16).reshape(NCORES, 128, NP, 3, 128)

    ws = np.stack([w5[:, :, 0, 2], w5[:, :, 1, 2], w5[:, :, 2, 2]], axis=0)
    ws = ws.reshape(3, O, C, NCORES, NP, 2)
    ws = ws.transpose(3, 2, 4, 0, 5, 1)       # [core, c, t, s, e, o]
    ws = np.ascontiguousarray(ws, dtype=BF16).reshape(NCORES, 64, NP, 3, 128)
    return wq, ws


def kernel(x, weight, bias, _trace=False, _trace_kwargs=None):
    from concourse.bass_utils import run_bass_kernel_spmd

    x = np.asarray(x, dtype=np.float32)
    weight = np.asarray(weight, dtype=np.float32)
    bias = np.asarray(bias, dtype=np.float32)

    nc = _get_nc()
    xs = _pack_x(x)
    wq, ws = _pack_w(weight)
    in_maps = [
        {"x": xs[r], "wq": wq[r], "ws": ws[r]} for r in range(NCORES)
    ]
    res = run_bass_kernel_spmd(
        nc, in_maps, list(range(NCORES)),
        trace=_trace, **(_trace_kwargs or {}),
    )
    # out[r]: [p=(e,o), t, b] bf16 -> [b, o, l=128r+2t+e]
    parts = []
    for r in range(NCORES):
        arr = res.results[r]["out"].astype(np.float32)
        arr = arr.reshape(2, O, NP, B).transpose(3, 1, 2, 0)  # [b, o, t, e]
        parts.append(arr.reshape(B, O, LC))
    out = np.concatenate(parts, axis=2).reshape(B, O, H, W)
    if np.any(bias):
        out = out + bias.reshape(1, O, H, W)
    if _trace:
        _CACHE["last_result"] = res
    return np.ascontiguousarray(out.astype(np.float32))


# revision 55
# speedup vs baseline: 1.1536x; 1.1536x over previous
"""Locally-connected layer (unshared 3x3 conv, torch-unfold semantics) on 8 trn2 cores.

out[b,o,y,x] = sum_{c,i,j} weight[o, c*9+i*3+j, y*32+x] * xpad[b, c, y+i, x+j]

Sharding: spatial over L — core r owns image rows [4r, 4r+4) (128 pixels).

Design (bf16, weights-stationary, N=128 moving, row-major slab):
  * Everything bf16 on the wire (tolerance 2e-2; measured error ~0.29%).
    PSUM accumulates fp32.
  * SBUF slab T1 [128, (row, b, w)] = [slab | slab shifted +1 col], host-built
    row-major and streamed per slab row, so compute starts as soon as row 0
    plus the first weight group land (PE warmup matmuls run meanwhile).
  * The 576-long contraction is reordered into 6 chunks; the stationary of
    each is a [K, 128] host-packed weight block covering BOTH pixels of a
    pair (cols m = 64*e + o).  Moving operand is x [K, N=128=(b, pix)] read
    as t1r[:, row, :, x0:x0+2].  PSUM [128, (b,pix)]: only the e==pix halves
    are read out.
      q0..q2: K=128  rows [c x (i=q,j=0) | c x (i=q,j=1)]
      s0..s2: K=64   rows  c x (s,2)  (ws blob, rows 0:64)
    All matmuls sit at row base 0 (mixed-base accumulation groups crash TRN2).
  * 6 matmuls / 6 ldweights per pixel pair (384 per core), 128-col
    stationaries, N=128 moving.  PSUM readout alternates vector/scalar by
    pair so the two engines touch different PSUM banks.
  * DMA budget and the throttled PE clock (~1.2 GHz sustained under 8-core
    load) are the binding resources: total moved = 3.34 (x) + 9.44 (w) +
    1.05 (out bf16) MB.  The weight stream owns the sync HWDGE queue
    (nothing ever blocks it); x and output DMAs ride the scalar queue.
  * Output bf16 in [psum-partition, pair, b] DRAM layout (contiguous DMA);
    host transposes to (B, O, H, W) fp32.
"""

import numpy as np
import ml_dtypes

BF16 = ml_dtypes.bfloat16

B, C, O, H, W, KS = 64, 64, 64, 32, 32, 3
L = H * W
NCORES = 8
RPC = H // NCORES            # image rows per core = 4
LC = RPC * W                 # pixels per core = 128
NP = LC // 2                 # pixel pairs per core = 64
HALO = RPC + 2               # 6 slab rows
WP = W + 2                   # padded width 34
PG = 8                       # pairs per weight DMA group
NG = NP // PG                # weight groups = 8

_CACHE = {}


def _build_nc():
    import concourse.bass as bass
    import concourse.bacc as bacc
    import concourse.tile as tile
    from concourse import mybir

    f32 = mybir.dt.float32
    bf16 = mybir.dt.bfloat16
    nc = bacc.Bacc(
        "TRN2", target_bir_lowering=False, debug=False, num_devices=NCORES
    )
    x_d = nc.dram_tensor("x", [128, HALO, B, WP], bf16, kind="ExternalInput")
    wq_d = nc.dram_tensor("wq", [128, NP, 3, 128], bf16, kind="ExternalInput")
    ws_d = nc.dram_tensor("ws", [64, NP, 3, 128], bf16, kind="ExternalInput")
    o_d = nc.dram_tensor("out", [128, NP, B], bf16, kind="ExternalOutput")

    with tile.TileContext(nc) as tc:
        with (
            tc.tile_pool(name="x1", bufs=1) as x1pool,
            tc.tile_pool(name="wq", bufs=6) as wpool,
            tc.tile_pool(name="ws", bufs=6) as spool,
            tc.tile_pool(name="orow", bufs=3) as opool,
            tc.tile_pool(name="ps", bufs=7, space=bass.MemorySpace.PSUM) as pspool,
            tc.tile_pool(name="psw", bufs=1, space=bass.MemorySpace.PSUM) as pswpool,
        ):
            t1 = x1pool.tile([128, HALO * B * WP], bf16)
            t1r = t1[:].rearrange("p (r b w) -> p r b w", r=HALO, b=B)
            # row-major slab: row 0 lands first (pair 0's q0 only needs it),
            # compute starts under the tail of the x transfer.
            nc.scalar.dma_start(t1r[:, 0:1], x_d[:, 0:1])
            nc.scalar.dma_start(t1r[:, 1:2], x_d[:, 1:2])
            nc.scalar.dma_start(t1r[:, 2:3], x_d[:, 2:3])
            nc.scalar.dma_start(t1r[:, 3:6], x_d[:, 3:6])

            # PE warmup: dummy matmuls while the first DMAs stream, so the
            # HAM clock gate is at 8/8 when the real matmul stream begins.
            scr = x1pool.tile([128, 256], bf16)
            nc.vector.memzero(scr[:])
            psw = pswpool.tile([64, 256], f32)
            for _ in range(28):
                nc.tensor.matmul(psw[:], scr[:, 0:64], scr[:], start=True, stop=True)

            # variable-size weight groups: small head (compute starts sooner)
            # and small tail (last output flushes sooner)
            sizes = [4, 4, 8, 8, 8, 8, 8, 8, 4, 4]
            t0g = 0
            for cnt in sizes:
                g0, g1 = t0g, t0g + cnt
                t0g = g1
                wt = wpool.tile([128, cnt, 3, 128], bf16)
                st = spool.tile([64, cnt, 3, 128], bf16)
                nc.sync.dma_start(wt[:], wq_d[:, g0:g1])
                nc.sync.dma_start(st[:], ws_d[:, g0:g1])
                orow = opool.tile([128, cnt, B], bf16)
                for tt in range(cnt):
                    t = g0 + tt
                    y, x0 = (2 * t) // W, (2 * t) % W
                    ps = pspool.tile([128, B, 2], f32)
                    for q in range(3):
                        nc.tensor.matmul(
                            ps[:], wt[:, tt, q, :],
                            t1r[:, y + q, :, x0 : x0 + 2],
                            start=(q == 0), stop=False,
                        )
                    for s in range(3):
                        nc.tensor.matmul(
                            ps[:], st[:, tt, s, :],
                            t1r[0:64, y + s, :, x0 + 2 : x0 + 4],
                            start=False, stop=(s == 2),
                        )
                    nc.vector.tensor_copy(orow[0:64, tt, :], ps[0:64, :, 0])
                    nc.vector.tensor_copy(orow[64:128, tt, :], ps[64:128, :, 1])
                nc.scalar.dma_start(o_d[:, g0:g1, :], orow[:])
    nc.compile()
    return nc


def _get_nc():
    if "nc" not in _CACHE:
        _CACHE["nc"] = _build_nc()
    return _CACHE["nc"]


def _pack_x(x):
    """Per core: [128, HALO, B, WP] bf16 = [slab | slab shifted +1 col],
    row-major so row blocks stream independently."""
    xpad = np.pad(x, ((0, 0), (0, 0), (1, 1), (1, 1)))
    xpad = np.ascontiguousarray(xpad.transpose(1, 0, 2, 3))  # [C, B, 34, 34]
    outs = []
    for r in range(NCORES):
        slab = xpad[:, :, RPC * r : RPC * r + HALO, :]       # [C, B, 6, 34]
        slab = slab.transpose(0, 2, 1, 3)                    # [C, 6, B, 34]
        up = np.zeros_like(slab)
        up[..., : WP - 1] = slab[..., 1:]
        t1 = np.concatenate([slab, up], axis=0).astype(BF16)
        outs.append(np.ascontiguousarray(t1))
    return outs


def _pack_w(weight):
    """Chunked-contraction weight blobs, already in SBUF layout.

    wq: [core, NG, p=(j, c), tt, q, m=(e, o)]   (pair chunks, shifts (q, j))
    ws: [core, NG, c, tt, s, m=(e, o)]          (singles, shifts (s, 2))
    """
    w5 = weight.reshape(O, C, KS, KS, L)
    low = np.stack([w5[:, :, 0, 0], w5[:, :, 1, 0], w5[:, :, 2, 0]], axis=0)
    up = np.stack([w5[:, :, 0, 1], w5[:, :, 1, 1], w5[:, :, 2, 1]], axis=0)
    wq = np.stack([low, up], axis=1)          # [q, j, O, C, L]
    wq = wq.reshape(3, 2, O, C, NCORES, NP, 2)
    # -> [core, j, c, t, q, e, o]
    wq = wq.transpose(4, 1, 3, 5, 0, 6, 2)
    wq = np.ascontiguousarray(wq, dtype=BF16).reshape(NCORES, 128, NP, 3, 128)

    ws = np.stack([w5[:, :, 0, 2], w5[:, :, 1, 2], w5[:, :, 2, 2]], axis=0)
    ws = ws.reshape(3, O, C, NCORES, NP, 2)
    ws = ws.transpose(3, 2, 4, 0, 5, 1)       # [core, c, t, s, e, o]
    ws = np.ascontiguousarray(ws, dtype=BF16).reshape(NCORES, 64, NP, 3, 128)
    return wq, ws


def kernel(x, weight, bias, _trace=False, _trace_kwargs=None):
    from concourse.bass_utils import run_bass_kernel_spmd

    x = np.asarray(x, dtype=np.float32)
    weight = np.asarray(weight, dtype=np.float32)
    bias = np.asarray(bias, dtype=np.float32)

    nc = _get_nc()
    xs = _pack_x(x)
    wq, ws = _pack_w(weight)
    in_maps = [
        {"x": xs[r], "wq": wq[r], "ws": ws[r]} for r in range(NCORES)
    ]
    res = run_bass_kernel_spmd(
        nc, in_maps, list(range(NCORES)),
        trace=_trace, **(_trace_kwargs or {}),
    )
    # out[r]: [p=(e,o), t, b] bf16 -> [b, o, l=128r+2t+e]
    parts = []
    for r in range(NCORES):
        arr = res.results[r]["out"].astype(np.float32)
        arr = arr.reshape(2, O, NP, B).transpose(3, 1, 2, 0)  # [b, o, t, e]
        parts.append(arr.reshape(B, O, LC))
    out = np.concatenate(parts, axis=2).reshape(B, O, H, W)
    if np.any(bias):
        out = out + bias.reshape(1, O, H, W)
    if _trace:
        _CACHE["last_result"] = res
    return np.ascontiguousarray(out.astype(np.float32))
